# revision 34
# baseline (speedup 1.0000x reference)
"""Trainium2 Bass kernel for fused ConvTranspose2d -> *0.5 -> global spatial mean.

Problem (hardcoded shapes):
  x      [64, 64, 128, 128] f32
  weight [64, 64, 3, 3]     f32  (ConvTranspose2d layout [Cin, Cout, kH, kW])
  bias   [64]               f32
  out    [64, 64, 1, 1]     f32
  stride=2, pad=1, output_padding=1 -> Hout=Wout=256.

Math: the conv-transpose + global mean collapses algebraically. With the
validity masks Vh/Vw (all ones except index [0,0]), the masked spatial sums
per kernel offset are expressible from four scalars per (b, c):
  T   = sum of the whole 128x128 image
  R0  = sum of row h=0
  C0  = sum of col w=0
  X   = x[b, c, 0, 0]
  acc[b,o] = sum_c  T*Wsum[c,o] - R0*Wrow0[c,o] - C0*Wcol0[c,o] + X*W00[c,o]
  out[b,o] = (acc / 65536 + bias[o]) * 0.5
where Wsum = sum over 3x3, Wrow0 = sum of kh=0 row, Wcol0 = sum of kw=0 col,
W00 = weight[c,o,0,0].  The 0.5/65536 scale is folded into the weight
features and 0.5 into the bias rank-1 matmul, so PSUM holds the final
output directly.

Sharding: data-parallel over batch across 8 cores (8 batches/core, 32 MiB of
x per core); weight/bias replicated; per-core output [8, 64]; host concat.

Per-core kernel: stream x with large multi-queue DMAs (sync/gpsimd/scalar
rings), reduce each chunk on the vector engine as it lands (partition
p = b2*64 + c over a 2-batch group), reduce partials straight into
zero-padded stat tensors [128, 8] (column = batch), then 5 tiny PE matmuls
(bias rank-1 first, then the four stat features) produce out [8, 64].
"""

import os
import sys

import numpy as np

_TRN_REPO = "/opt/trn_rl_repo"
if _TRN_REPO not in sys.path and os.path.isdir(_TRN_REPO):
    sys.path.insert(0, _TRN_REPO)

import concourse.bass as bass
import concourse.tile as tile
from concourse import bacc, mybir
from concourse.bass_utils import run_bass_kernel_spmd

B, CIN, HIN, WIN = 64, 64, 128, 128
COUT, K = 64, 3
N_CORES = 8
B_PER_CORE = B // N_CORES          # 8
HW = HIN * WIN                     # 16384
N_DBL = B_PER_CORE // 2            # 4 double-batch tiles per core
FP32 = mybir.dt.float32
AX = mybir.AxisListType
ADD = mybir.AluOpType.add
SCALE = 0.5 / float(HW * 4)        # 0.5 / (Hout*Wout)

# chunk layout (elements per partition) per double-batch group: 2 MiB chunks
# for the steady stream; the last group tapers so the post-stream serial
# chain ends on a small final reduce.
CHUNKS = {
    0: [4096] * 4,
    1: [4096] * 4,
    2: [4096] * 4,
    3: [4096, 4096, 4096, 2048, 1024, 512, 512],
}

_CACHE = {}


def _build_bass():
    nc = bacc.Bacc("TRN2", target_bir_lowering=False, debug=False,
                   num_devices=N_CORES, num_swdge_queues=2)

    x_d = nc.dram_tensor("x", [B_PER_CORE, CIN, HIN, WIN], FP32, kind="ExternalInput")
    w_d = nc.dram_tensor("weight", [CIN, COUT, K, K], FP32, kind="ExternalInput")
    b_d = nc.dram_tensor("bias", [COUT], FP32, kind="ExternalInput")
    o_d = nc.dram_tensor("out", [B_PER_CORE, COUT], FP32, kind="ExternalOutput")

    with tile.TileContext(nc) as tc:
        with (
            tc.tile_pool(name="const", bufs=1) as const,
            tc.tile_pool(name="xin", bufs=8) as xin,
            tc.tile_pool(name="tmp", bufs=8) as tmp,
            tc.tile_pool(name="psum", bufs=1, space="PSUM") as psum,
        ):
            # ---- weight / bias prep (tiny; on the ACT HWDGE ring so the
            # gpsimd queue starts with the x stream) ----
            w_sb = const.tile([CIN, COUT * K * K], FP32, tag="w_sb")
            nc.scalar.dma_start(w_sb[:], w_d[:].rearrange("c o kh kw -> c (o kh kw)"))
            bias_sb = const.tile([1, COUT], FP32, tag="bias_sb")
            nc.scalar.dma_start(bias_sb[:], b_d[:].rearrange("(a o) -> a o", a=1))

            w3 = w_sb[:].rearrange("c (o e) -> c o e", e=K * K)      # [64,64,9]
            w4 = w_sb[:].rearrange("c (o kh kw) -> c o kh kw", kh=K, kw=K)

            wsum = const.tile([CIN, COUT], FP32, tag="wsum")
            nc.vector.tensor_reduce(wsum[:], w3, axis=AX.X, op=ADD)
            wrow0 = const.tile([CIN, COUT], FP32, tag="wrow0")
            nc.vector.tensor_reduce(wrow0[:], w4[:, :, 0:1, :], axis=AX.XY, op=ADD)
            wcol0 = const.tile([CIN, COUT], FP32, tag="wcol0")
            nc.vector.tensor_reduce(wcol0[:], w4[:, :, :, 0:1], axis=AX.XY, op=ADD)

            # Weight-feature matrices [128, 64]: channel feature duplicated
            # across both batch halves (partition p = b2*64 + c); sign and
            # the final 0.5/65536 output scale folded in.
            wf_s = const.tile([128, COUT], FP32, tag="wf_s")
            wf_r = const.tile([128, COUT], FP32, tag="wf_r")
            wf_c = const.tile([128, COUT], FP32, tag="wf_c")
            wf_x = const.tile([128, COUT], FP32, tag="wf_x")
            for half in range(2):
                dst = slice(half * 64, half * 64 + 64)
                nc.scalar.mul(wf_s[dst, :], wsum[:], SCALE)
                nc.scalar.mul(wf_r[dst, :], wrow0[:], -SCALE)
                nc.scalar.mul(wf_c[dst, :], wcol0[:], -SCALE)
                nc.scalar.mul(wf_x[dst, :], w3[:, :, 0:1], SCALE)

            # ---- stat tensors [128, 8] (zero-padded; col = global batch) ----
            st_s = const.tile([128, B_PER_CORE], FP32, tag="st_s")
            st_r = const.tile([128, B_PER_CORE], FP32, tag="st_r")
            st_c = const.tile([128, B_PER_CORE], FP32, tag="st_c")
            st_x = const.tile([128, B_PER_CORE], FP32, tag="st_x")
            for st in (st_s, st_r, st_c, st_x):
                nc.vector.memset(st[:], 0.0)
            ones = const.tile([1, B_PER_CORE], FP32, tag="ones")
            nc.vector.memset(ones[:], 0.5)

            # ---- main loop: stream x, reduce chunks as they land ----
            # Whole-chunk processing alternates between the vector engine
            # (tensor_reduce) and the scalar engine (activation Copy with
            # accum_out), halving the reduce path. Each x tile slot is read
            # by exactly ONE engine so the slot-reuse DMA needs just one
            # wait and the gpsimd issue stream never stalls on two engines.
            max_chunks = max(len(v) for v in CHUNKS.values())
            act_scr = const.tile([128, 4096], FP32, tag="act_scr")
            act_scr_c = const.tile([128, 64], FP32, tag="act_scr_c")
            kglob = 0
            for d in range(N_DBL):
                chunks = CHUNKS[d]
                src = x_d[2 * d : 2 * d + 2].rearrange("b c h w -> (b c) (h w)")
                part_s = tmp.tile([128, max_chunks], FP32, tag="part_s")
                part_c = tmp.tile([128, max_chunks], FP32, tag="part_c")
                off = 0
                for j, csz in enumerate(chunks):
                    ct = xin.tile([128, csz], FP32, tag="ct")
                    sl = slice(off, off + csz)
                    nc.gpsimd.dma_start(ct[:], src[:, sl])
                    rows = csz // WIN
                    ch = ct[:].rearrange("p (h w) -> p h w", w=WIN)
                    # final two chunks stay on DVE so the post-stream
                    # chain has no cross-engine hop
                    last_grp = d == N_DBL - 1
                    on_dve = kglob % 2 == 0 or (last_grp and j >= len(chunks) - 2)
                    kglob += 1
                    if on_dve:
                        nc.vector.tensor_reduce(
                            part_s[:, j : j + 1], ct[:], axis=AX.X, op=ADD
                        )
                        nc.vector.tensor_reduce(
                            part_c[:, j : j + 1], ch[:, :, 0:1], axis=AX.XY, op=ADD
                        )
                    else:
                        nc.scalar.activation(
                            act_scr[:, 0:csz],
                            ct[:],
                            mybir.ActivationFunctionType.Copy,
                            accum_out=part_s[:, j : j + 1],
                        )
                        nc.scalar.activation(
                            act_scr_c[:, 0:rows],
                            ch[:, :, 0],
                            mybir.ActivationFunctionType.Copy,
                            accum_out=part_c[:, j : j + 1],
                        )
                    if j == 0:
                        # row-0 sum and corner pixel (both inside chunk 0;
                        # chunk 0 is always a DVE chunk)
                        assert on_dve
                        for b2 in range(2):
                            p = slice(b2 * 64, b2 * 64 + 64)
                            col = slice(2 * d + b2, 2 * d + b2 + 1)
                            nc.vector.tensor_reduce(
                                st_r[p, col], ct[p, 0:WIN], axis=AX.X, op=ADD
                            )
                            nc.vector.tensor_copy(st_x[p, col], ct[p, 0:1])
                    off += csz

                # combine chunk partials straight into the stat slots
                nch = len(chunks)
                for b2 in range(2):
                    p = slice(b2 * 64, b2 * 64 + 64)
                    col = slice(2 * d + b2, 2 * d + b2 + 1)
                    nc.vector.tensor_reduce(
                        st_s[p, col], part_s[p, 0:nch], axis=AX.X, op=ADD
                    )
                    nc.vector.tensor_reduce(
                        st_c[p, col], part_c[p, 0:nch], axis=AX.X, op=ADD
                    )

            # ---- contraction on PE; PSUM holds the final [8, 64] output ----
            # Order: ready-early operands first so only the S matmul trails
            # the final chunk.
            acc = psum.tile([B_PER_CORE, COUT], FP32, tag="acc")
            nc.tensor.matmul(acc[:], ones[:], bias_sb[:], start=True, stop=False)
            nc.tensor.matmul(acc[:], st_r[:], wf_r[:], start=False, stop=False)
            nc.tensor.matmul(acc[:], st_x[:], wf_x[:], start=False, stop=False)
            nc.tensor.matmul(acc[:], st_c[:], wf_c[:], start=False, stop=False)
            nc.tensor.matmul(acc[:], st_s[:], wf_s[:], start=False, stop=True)

            out_sb = const.tile([B_PER_CORE, COUT], FP32, tag="out_sb")
            nc.vector.tensor_copy(out_sb[:], acc[:])
            nc.sync.dma_start(o_d[:], out_sb[:])

    nc.compile()
    return nc


def _get_nc():
    if "nc" not in _CACHE:
        _CACHE["nc"] = _build_bass()
    return _CACHE["nc"]


def kernel(x, weight, bias, _trace=False, _tmpdir=None):
    x = np.ascontiguousarray(np.asarray(x, dtype=np.float32))
    weight = np.ascontiguousarray(np.asarray(weight, dtype=np.float32))
    bias = np.ascontiguousarray(np.asarray(bias, dtype=np.float32))
    assert x.shape == (B, CIN, HIN, WIN), x.shape

    nc = _get_nc()
    in_maps = [
        {
            "x": x[i * B_PER_CORE : (i + 1) * B_PER_CORE],
            "weight": weight,
            "bias": bias,
        }
        for i in range(N_CORES)
    ]
    res = run_bass_kernel_spmd(
        nc, in_maps, list(range(N_CORES)), trace=_trace, tmpdir=_tmpdir
    )
    _CACHE["last_results"] = res
    out = np.concatenate([res.results[i]["out"] for i in range(N_CORES)], axis=0)
    return out.reshape(B, COUT, 1, 1).astype(np.float32)
